# revision 12
# baseline (speedup 1.0000x reference)
"""Bass/Trainium2 kernel for nn_ContrastiveLoss_18502719111626.

Reference math:
    mask_i = (sum_d latent[i,d] != 0)
    ln     = latent / max(||latent_i||, 1e-8)
    total  = einsum('i,ij,j->', mask, ln @ ln.T, mask) - sum(mask)
    out    = 0.01 * total / (2 * N)

Key identity: einsum('i,ij,j->', m, ln@ln.T, m) == ||sum_i m_i * ln_i||^2,
so the N x N similarity matrix is never needed. Each core streams its
1024-row shard once (memory-roofline), producing a 64-dim weighted
column sum s_c and a mask count c_c. Host combines:
    total = ||sum_c s_c||^2 - sum_c c_c.

Per-core dataflow (shard [1024, 64] f32):
    X[128, 512] sbuf, col-group g = shard rows g*128..g*128+127 (8 DMAs)
    ss8[p,g] = sum_d X[p, g*64+d]^2    (8 ScalarE Square ops w/ accum_out)
    rs8[p,g] = sum_d X[p, g*64+d]      (1 VectorE reduce over [128,8,64])
    scale8 = (rs8 != 0) / max(sqrt(ss8), eps)
    psum_s[1,64] += scale8[:,g].T @ X[:,g*64:(g+1)*64]   (8 accumulating matmuls)
    psum_c[1,1]  = cnt_per_partition.T @ ones            (1 matmul)
    partials[1,65] = [s | cnt] -> DRAM
"""

import numpy as np

N = 8192
D = 64
NCORES = 8
ROWS = N // NCORES  # 1024 rows per core
GROUPS = ROWS // 128  # 8 column-groups of the sbuf tile
COF1 = 0.01
EPS = 1e-8

_prog = None


def _build(n_in_dmas=8):
    import concourse.bacc as bacc
    import concourse.mybir as mybir
    import concourse.tile as tile

    f32 = mybir.dt.float32
    AF = mybir.ActivationFunctionType
    ALU = mybir.AluOpType

    # Bacc (not plain Bass): its compile() runs generate_event_semaphores,
    # which splits multi-sem sync waits into EventSemaphore instructions --
    # walrus rejects >1 wait per instruction.
    nc = bacc.Bacc(None)
    x_in = nc.declare_dram_parameter("latent", [ROWS, D], f32, isOutput=False)
    out_p = nc.declare_dram_parameter("partials", [1, D + 1], f32, isOutput=True)

    with tile.TileContext(nc) as tc:
        with (
            tc.tile_pool(name="sbuf", bufs=1) as pool,
            tc.tile_pool(name="psum", bufs=1, space="PSUM") as psum_pool,
        ):
            X = pool.tile([128, GROUPS * D], f32)
            # Column-group g holds shard rows g*128..g*128+127 (256B
            # contiguous per partition). Few dma_starts: the kernel-tail
            # drain and the result-store DMA have limited sync-wait slots,
            # so total DMA-queue usage must stay small.
            gs = GROUPS // n_in_dmas  # groups per dma_start
            for c in range(n_in_dmas):
                nc.sync.dma_start(
                    out=X[:, c * gs * D : (c + 1) * gs * D].rearrange(
                        "p (g d) -> p g d", g=gs
                    ),
                    in_=x_in[c * gs * 128 : (c + 1) * gs * 128, :].rearrange(
                        "(g p) d -> p g d", p=128
                    ),
                )

            ones = pool.tile([128, 1], f32)
            nc.vector.memset(ones[:], 1.0)

            # Row sum-of-squares per group on ScalarE (frees VectorE).
            sq = pool.tile([128, GROUPS * D], f32)
            ss8 = pool.tile([128, GROUPS], f32)
            for g in range(GROUPS):
                nc.scalar.activation(
                    out=sq[:, g * D : (g + 1) * D],
                    in_=X[:, g * D : (g + 1) * D],
                    func=AF.Square,
                    accum_out=ss8[:, g : g + 1],
                )

            # Row sums per group on VectorE via tensor_scalar accum (a single
            # big reduce would need a sync-wait per DMA queue and overflow the
            # instruction's sync-wait slots; per-group ops wait on one DMA each).
            xcopy = pool.tile([128, GROUPS * D], f32)
            rs8 = pool.tile([128, GROUPS], f32)
            for g in range(GROUPS):
                nc.vector.tensor_scalar(
                    xcopy[:, g * D : (g + 1) * D],
                    X[:, g * D : (g + 1) * D],
                    1.0, 0.0,
                    op0=ALU.mult, op1=ALU.add,
                    accum_out=rs8[:, g : g + 1],
                )

            # scale = mask / max(sqrt(ss), eps); cnt via accum of mask.
            norm = pool.tile([128, GROUPS], f32)
            nc.scalar.sqrt(norm[:], ss8[:])
            normc = pool.tile([128, GROUPS], f32)
            nc.vector.tensor_scalar_max(normc[:], norm[:], EPS)
            inv = pool.tile([128, GROUPS], f32)
            nc.vector.reciprocal(inv[:], normc[:])
            mask = pool.tile([128, GROUPS], f32)
            cntp = pool.tile([128, 1], f32)
            nc.vector.tensor_scalar(
                mask[:], rs8[:], 0.0, 0.0,
                op0=ALU.not_equal, op1=ALU.add, accum_out=cntp[:],
            )
            scale8 = pool.tile([128, GROUPS], f32)
            nc.vector.tensor_mul(scale8[:], inv[:], mask[:])

            # s[1,64]: weighted column sums, accumulated in PSUM over groups.
            psum_s = psum_pool.tile([1, D], f32)
            for g in range(GROUPS):
                nc.tensor.matmul(
                    psum_s[:],
                    scale8[:, g : g + 1],
                    xcopy[:, g * D : (g + 1) * D],
                    start=(g == 0),
                    stop=(g == GROUPS - 1),
                )
            psum_c = psum_pool.tile([1, 1], f32)
            nc.tensor.matmul(psum_c[:], cntp[:], ones[:], start=True, stop=True)

            res = pool.tile([1, D + 1], f32)
            nc.vector.tensor_copy(res[:, :D], psum_s[:])
            nc.vector.tensor_copy(res[:, D : D + 1], psum_c[:])
            nc.sync.dma_start(out=out_p[:, :], in_=res[:])

    nc.compile()
    return nc


def _run_spmd(latent, trace=False, **kw):
    from concourse.bass_utils import run_bass_kernel_spmd

    global _prog
    if _prog is None:
        _prog = _build()
    in_maps = [
        {"latent": np.ascontiguousarray(latent[c * ROWS : (c + 1) * ROWS])}
        for c in range(NCORES)
    ]
    return run_bass_kernel_spmd(_prog, in_maps, list(range(NCORES)), trace=trace, **kw)


def _combine(results):
    parts = np.stack([results[c]["partials"][0] for c in range(NCORES)])  # [8, 65]
    s = parts[:, :D].astype(np.float64).sum(axis=0)
    cnt = parts[:, D].astype(np.float64).sum()
    total = float(s @ s - cnt)
    return np.asarray(COF1 * total / (2.0 * N), dtype=np.float32)


def kernel(latent):
    latent = np.asarray(latent, dtype=np.float32)
    assert latent.shape == (N, D)
    return _combine(_run_spmd(latent).results)


# revision 16
# speedup vs baseline: 1.0674x; 1.0674x over previous
"""Bass/Trainium2 kernel for nn_ContrastiveLoss_18502719111626.

Reference math:
    mask_i = (sum_d latent[i,d] != 0)
    ln     = latent / max(||latent_i||, 1e-8)
    total  = einsum('i,ij,j->', mask, ln @ ln.T, mask) - sum(mask)
    out    = 0.01 * total / (2 * N)

Key identity: einsum('i,ij,j->', m, ln@ln.T, m) == ||sum_i m_i * ln_i||^2,
so the N x N similarity matrix is never needed. Each core streams its
1024-row shard once (memory-roofline), producing a 64-dim weighted
column sum s_c and a mask count c_c. Host combines:
    total = ||sum_c s_c||^2 - sum_c c_c.

Per-core dataflow (shard [1024, 64] f32):
    X[128, 512] sbuf, col-group g = shard rows g*128..g*128+127 (8 DMAs)
    ss8[p,g] = sum_d X[p, g*64+d]^2    (8 ScalarE Square ops w/ accum_out)
    rs8[p,g] = sum_d X[p, g*64+d]      (1 VectorE reduce over [128,8,64])
    scale8 = (rs8 != 0) / max(sqrt(ss8), eps)
    psum_s[1,64] += scale8[:,g].T @ X[:,g*64:(g+1)*64]   (8 accumulating matmuls)
    psum_c[1,1]  = cnt_per_partition.T @ ones            (1 matmul)
    partials[1,65] = [s | cnt] -> DRAM
"""

import numpy as np

N = 8192
D = 64
NCORES = 8
ROWS = N // NCORES  # 1024 rows per core
GROUPS = ROWS // 128  # 8 column-groups of the sbuf tile
COF1 = 0.01
EPS = 1e-8

_prog = None


def _build(n_in_dmas=8):
    import concourse.bacc as bacc
    import concourse.mybir as mybir
    import concourse.tile as tile

    f32 = mybir.dt.float32
    AF = mybir.ActivationFunctionType
    ALU = mybir.AluOpType

    # Bacc (not plain Bass): its compile() runs generate_event_semaphores,
    # which splits multi-sem sync waits into EventSemaphore instructions --
    # walrus rejects >1 wait per instruction.
    nc = bacc.Bacc(None)
    x_in = nc.declare_dram_parameter("latent", [ROWS, D], f32, isOutput=False)
    out_p = nc.declare_dram_parameter("partials", [1, D + 1], f32, isOutput=True)

    with tile.TileContext(nc) as tc:
        with (
            tc.tile_pool(name="sbuf", bufs=1) as pool,
            tc.tile_pool(name="psum", bufs=1, space="PSUM") as psum_pool,
        ):
            X = pool.tile([128, GROUPS * D], f32)
            # Column-group g holds shard rows g*128..g*128+127 (256B
            # contiguous per partition). Few dma_starts: the kernel-tail
            # drain and the result-store DMA have limited sync-wait slots,
            # so total DMA-queue usage must stay small.
            gs = GROUPS // n_in_dmas  # groups per dma_start
            for c in range(n_in_dmas):
                nc.sync.dma_start(
                    out=X[:, c * gs * D : (c + 1) * gs * D].rearrange(
                        "p (g d) -> p g d", g=gs
                    ),
                    in_=x_in[c * gs * 128 : (c + 1) * gs * 128, :].rearrange(
                        "(g p) d -> p g d", p=128
                    ),
                )

            ones = pool.tile([128, 1], f32)
            nc.vector.memset(ones[:], 1.0)

            # Dummy sqrt as ScalarE's first instruction: pulls in the
            # "sqrt_and_others" activation table (which also contains
            # square), so only one ACT_TABLE_LOAD happens, early, instead
            # of a second 1.3us load mid-kernel right before the real sqrt.
            warm = pool.tile([128, 1], f32)
            nc.scalar.sqrt(warm[:], ones[:])

            # Row sum-of-squares per group on ScalarE (frees VectorE).
            sq = pool.tile([128, GROUPS * D], f32)
            ss8 = pool.tile([128, GROUPS], f32)
            for g in range(GROUPS):
                nc.scalar.activation(
                    out=sq[:, g * D : (g + 1) * D],
                    in_=X[:, g * D : (g + 1) * D],
                    func=AF.Square,
                    accum_out=ss8[:, g : g + 1],
                )

            # Row sums per group on VectorE via tensor_scalar accum (a single
            # big reduce would need a sync-wait per DMA queue and overflow the
            # instruction's sync-wait slots; per-group ops wait on one DMA each).
            xcopy = pool.tile([128, GROUPS * D], f32)
            rs8 = pool.tile([128, GROUPS], f32)
            for g in range(GROUPS):
                nc.vector.tensor_scalar(
                    xcopy[:, g * D : (g + 1) * D],
                    X[:, g * D : (g + 1) * D],
                    1.0, 0.0,
                    op0=ALU.mult, op1=ALU.add,
                    accum_out=rs8[:, g : g + 1],
                )

            # scale = (rs != 0) / max(sqrt(ss), eps); cnt via accum of mask.
            # max(sqrt(ss), eps) == sqrt(max(ss, eps^2)) since ss >= 0.
            ssc = pool.tile([128, GROUPS], f32)
            nc.vector.tensor_scalar_max(ssc[:], ss8[:], EPS * EPS)
            norm = pool.tile([128, GROUPS], f32)
            nc.scalar.sqrt(norm[:], ssc[:])
            mask = pool.tile([128, GROUPS], f32)
            cntp = pool.tile([128, 1], f32)
            nc.vector.tensor_scalar(
                mask[:], rs8[:], 0.0, 0.0,
                op0=ALU.not_equal, op1=ALU.add, accum_out=cntp[:],
            )
            inv = pool.tile([128, GROUPS], f32)
            nc.vector.reciprocal(inv[:], norm[:])
            scale8 = pool.tile([128, GROUPS], f32)
            nc.vector.tensor_mul(scale8[:], inv[:], mask[:])

            # s[1,64]: weighted column sums, accumulated in PSUM over groups.
            psum_s = psum_pool.tile([1, D], f32)
            for g in range(GROUPS):
                nc.tensor.matmul(
                    psum_s[:],
                    scale8[:, g : g + 1],
                    xcopy[:, g * D : (g + 1) * D],
                    start=(g == 0),
                    stop=(g == GROUPS - 1),
                )
            psum_c = psum_pool.tile([1, 1], f32)
            nc.tensor.matmul(psum_c[:], cntp[:], ones[:], start=True, stop=True)

            res = pool.tile([1, D + 1], f32)
            nc.vector.tensor_copy(res[:, :D], psum_s[:])
            nc.vector.tensor_copy(res[:, D : D + 1], psum_c[:])
            nc.sync.dma_start(out=out_p[:, :], in_=res[:])

    nc.compile()
    return nc


def _run_spmd(latent, trace=False, **kw):
    from concourse.bass_utils import run_bass_kernel_spmd

    global _prog
    if _prog is None:
        _prog = _build()
    in_maps = [
        {"latent": np.ascontiguousarray(latent[c * ROWS : (c + 1) * ROWS])}
        for c in range(NCORES)
    ]
    return run_bass_kernel_spmd(_prog, in_maps, list(range(NCORES)), trace=trace, **kw)


def _combine(results):
    parts = np.stack([results[c]["partials"][0] for c in range(NCORES)])  # [8, 65]
    s = parts[:, :D].astype(np.float64).sum(axis=0)
    cnt = parts[:, D].astype(np.float64).sum()
    total = float(s @ s - cnt)
    return np.asarray(COF1 * total / (2.0 * N), dtype=np.float32)


def kernel(latent):
    latent = np.asarray(latent, dtype=np.float32)
    assert latent.shape == (N, D)
    return _combine(_run_spmd(latent).results)
